# revision 1
# baseline (speedup 1.0000x reference)
"""ColBERT contrastive forward, distributed over 8 Trainium2 NeuronCores.

Sharding strategy (hardcoded for B=32, N=32, LQ=128, LD=512, H=768, C=128):
  - Queries + positive docs: data-parallel over the batch dim (4 rows/core).
    The whole pos MaxSim path is local to each core.
  - Negative docs: sharded 4 docs/core for the embedding projection; the
    small query embeddings (32 x 127 x 128) are all-gathered so every core
    computes the [32, 4, 127, 511] slice of the neg MaxSim (columns of the
    BxN score matrix). Scores are then all-gathered (tiny) so the loss is
    computed on-device.
"""

import numpy as np
import jax
import jax.numpy as jnp
from jax.sharding import Mesh, PartitionSpec as P
from jax.experimental.shard_map import shard_map
from functools import partial

PAD_ID = 0
CLS_ID = 101
TEMPERATURE = 0.02
EPS = 1e-12

NCORES = 8
B, N, LQ, LD, H, C = 32, 32, 128, 512, 768, 128


def _get_embedding(hidden, input_ids, w, b):
    # hidden [n, L, H] f32, input_ids [n, L] int32
    e = jnp.einsum("bld,cd->blc", hidden[:, 1:], w) + b
    keep = jnp.logical_not((input_ids == PAD_ID) | (input_ids == CLS_ID))
    e = e * keep[:, 1:, None].astype(e.dtype)
    n = jnp.linalg.norm(e, axis=-1, keepdims=True)
    return e / jnp.maximum(n, EPS)


def _maxsim_chunked(q_all, qlen_all, nd):
    # q_all [B, LQ-1, C], nd [n_loc, LD-1, C] -> scores [B, n_loc]
    # Chunk over global queries to bound the materialized Gram tile.
    def body(carry, qc):
        qs, ql = qc  # qs [4, LQ-1, C], ql [4]
        tok = jnp.einsum("bqc,nkc->bnqk", qs, nd)  # [4, n_loc, LQ-1, LD-1]
        s = tok.max(-1).sum(-1) / ql[:, None]
        return carry, s

    nchunk = 8
    qs = q_all.reshape(nchunk, B // nchunk, LQ - 1, C)
    qls = qlen_all.reshape(nchunk, B // nchunk)
    _, out = jax.lax.scan(body, 0, (qs, qls))
    return out.reshape(B, nd.shape[0])


def _shard_fn(qh, qid, qmask, ph, pid, nh, nid, w, b):
    # per-core shapes: qh [4,LQ,H], ph [4,LD,H], nh [4,LD,H]
    q = _get_embedding(qh, qid, w, b)      # [4, LQ-1, C]
    pd = _get_embedding(ph, pid, w, b)     # [4, LD-1, C]
    nd = _get_embedding(nh, nid, w, b)     # [4, LD-1, C]

    qlen = qmask[:, 1:].sum(-1).astype(q.dtype)  # [4]

    # pos path: fully local
    pos_tok = jnp.einsum("bqc,bkc->bqk", q, pd)        # [4, LQ-1, LD-1]
    pos_score = pos_tok.max(-1).sum(-1) / qlen          # [4]

    # neg path: all-gather the (small) query embeddings, compute the
    # [B, n_loc] column block of the score matrix against local neg docs.
    q_all = jax.lax.all_gather(q, "c", axis=0, tiled=True)        # [B, LQ-1, C]
    qlen_all = jax.lax.all_gather(qlen, "c", axis=0, tiled=True)  # [B]
    neg_block = _maxsim_chunked(q_all, qlen_all, nd)               # [B, 4]

    # gather full score matrix (tiny) and compute the loss on every core
    neg_all = jax.lax.all_gather(neg_block, "c", axis=1, tiled=True)  # [B, N]
    pos_all = jax.lax.all_gather(pos_score, "c", axis=0, tiled=True)  # [B]

    logits = jnp.concatenate([pos_all[:, None], neg_all], axis=1) / TEMPERATURE
    loss = -jax.nn.log_softmax(logits, axis=-1)[:, 0].mean()
    return pos_score, loss[None]


_compiled = None


def _build():
    global _compiled
    if _compiled is not None:
        return _compiled
    devs = jax.devices()[:NCORES]
    mesh = Mesh(np.array(devs), ("c",))
    sh = P("c")
    rep = P()
    fn = shard_map(
        _shard_fn,
        mesh=mesh,
        in_specs=(sh, sh, sh, sh, sh, sh, sh, rep, rep),
        out_specs=(sh, sh),
        check_rep=False,
    )
    _compiled = jax.jit(fn)
    return _compiled


def kernel(
    query_hidden,
    query_input_ids,
    query_attention_mask,
    pos_doc_hidden,
    pos_doc_input_ids,
    pos_doc_attention_mask,
    neg_doc_hidden,
    neg_doc_input_ids,
    neg_doc_attention_mask,
    linear_w,
    linear_b,
):
    fn = _build()
    qh = np.asarray(query_hidden, dtype=np.float32)
    ph = np.asarray(pos_doc_hidden, dtype=np.float32)
    nh = np.asarray(neg_doc_hidden, dtype=np.float32)
    qid = np.asarray(query_input_ids, dtype=np.int32)
    pid = np.asarray(pos_doc_input_ids, dtype=np.int32)
    nid = np.asarray(neg_doc_input_ids, dtype=np.int32)
    qmask = np.asarray(query_attention_mask, dtype=np.int32)
    w = np.asarray(linear_w, dtype=np.float32)
    b = np.asarray(linear_b, dtype=np.float32)

    pos, loss = fn(qh, qid, qmask, ph, pid, nh, nid, w, b)
    pos = np.asarray(pos)          # [B]
    loss = np.asarray(loss)[0]     # loss replicated; all entries identical
    return pos, np.float32(loss)


# revision 3
# speedup vs baseline: 1.2095x; 1.2095x over previous
"""ColBERT contrastive forward, distributed over 8 Trainium2 NeuronCores.

Sharding strategy (hardcoded for B=32, N=32, LQ=128, LD=512, H=768, C=128):
  - Queries + positive docs: data-parallel over the batch dim (4 rows/core).
    The whole pos MaxSim path is local to each core.
  - Negative docs: sharded 4 docs/core for the embedding projection; the
    small query embeddings (32 x 127 x 128) are all-gathered so every core
    computes the [32, 4, 127, 511] slice of the neg MaxSim (columns of the
    BxN score matrix). Scores are then all-gathered (tiny) so the loss is
    computed on-device.
"""

import numpy as np
import jax
import jax.numpy as jnp
from jax.sharding import Mesh, PartitionSpec as P
from jax.experimental.shard_map import shard_map
from functools import partial

PAD_ID = 0
CLS_ID = 101
TEMPERATURE = 0.02
EPS = 1e-12

NCORES = 8
B, N, LQ, LD, H, C = 32, 32, 128, 512, 768, 128


def _get_embedding(hidden, input_ids, w, b):
    # hidden [n, L, H] f32, input_ids [n, L] int32
    # bf16 matmul (f32 accumulate) - TensorE runs 4x faster than f32.
    e = jnp.einsum(
        "bld,cd->blc",
        hidden[:, 1:].astype(jnp.bfloat16),
        w.astype(jnp.bfloat16),
        preferred_element_type=jnp.float32,
    ) + b
    keep = jnp.logical_not((input_ids == PAD_ID) | (input_ids == CLS_ID))
    e = e * keep[:, 1:, None].astype(e.dtype)
    n = jnp.linalg.norm(e, axis=-1, keepdims=True)
    return e / jnp.maximum(n, EPS)


def _maxsim_chunked(q_all, qlen_all, nd):
    # q_all [B, LQ-1, C] bf16, nd [n_loc, LD-1, C] bf16 -> scores [B, n_loc]
    # Chunk over global queries to bound the materialized Gram tile; keep the
    # Gram in bf16 to halve its HBM round-trip, reduce max in bf16, sum in f32.
    def body(carry, qc):
        qs, ql = qc  # qs [4, LQ-1, C], ql [4]
        tok = jnp.einsum(
            "bqc,nkc->bnqk", qs, nd, preferred_element_type=jnp.bfloat16
        )  # [4, n_loc, LQ-1, LD-1] bf16
        s = tok.max(-1).astype(jnp.float32).sum(-1) / ql[:, None]
        return carry, s

    nchunk = 8
    qs = q_all.reshape(nchunk, B // nchunk, LQ - 1, C)
    qls = qlen_all.reshape(nchunk, B // nchunk)
    _, out = jax.lax.scan(body, 0, (qs, qls))
    return out.reshape(B, nd.shape[0])


def _shard_fn(qh, qid, qmask, ph, pid, nh, nid, w, b):
    # per-core shapes: qh [4,LQ,H], ph [4,LD,H], nh [4,LD,H]
    q = _get_embedding(qh, qid, w, b)      # [4, LQ-1, C]
    pd = _get_embedding(ph, pid, w, b)     # [4, LD-1, C]
    nd = _get_embedding(nh, nid, w, b)     # [4, LD-1, C]

    qlen = qmask[:, 1:].sum(-1).astype(jnp.float32)  # [4]

    q = q.astype(jnp.bfloat16)
    pd = pd.astype(jnp.bfloat16)
    nd = nd.astype(jnp.bfloat16)

    # pos path: fully local
    pos_tok = jnp.einsum(
        "bqc,bkc->bqk", q, pd, preferred_element_type=jnp.bfloat16
    )  # [4, LQ-1, LD-1]
    pos_score = pos_tok.max(-1).astype(jnp.float32).sum(-1) / qlen  # [4]

    # neg path: all-gather the (small, bf16) query embeddings, compute the
    # [B, n_loc] column block of the score matrix against local neg docs.
    q_all = jax.lax.all_gather(q, "c", axis=0, tiled=True)        # [B, LQ-1, C]
    qlen_all = jax.lax.all_gather(qlen, "c", axis=0, tiled=True)  # [B]
    neg_block = _maxsim_chunked(q_all, qlen_all, nd)               # [B, 4]

    # gather full score matrix (tiny) and compute the loss on every core
    neg_all = jax.lax.all_gather(neg_block, "c", axis=1, tiled=True)  # [B, N]
    pos_all = jax.lax.all_gather(pos_score, "c", axis=0, tiled=True)  # [B]

    logits = jnp.concatenate([pos_all[:, None], neg_all], axis=1) / TEMPERATURE
    loss = -jax.nn.log_softmax(logits, axis=-1)[:, 0].mean()
    return pos_score, loss[None]


_compiled = None


def _build():
    global _compiled
    if _compiled is not None:
        return _compiled
    devs = jax.devices()[:NCORES]
    mesh = Mesh(np.array(devs), ("c",))
    sh = P("c")
    rep = P()
    fn = shard_map(
        _shard_fn,
        mesh=mesh,
        in_specs=(sh, sh, sh, sh, sh, sh, sh, rep, rep),
        out_specs=(sh, sh),
        check_rep=False,
    )
    _compiled = jax.jit(fn)
    return _compiled


def kernel(
    query_hidden,
    query_input_ids,
    query_attention_mask,
    pos_doc_hidden,
    pos_doc_input_ids,
    pos_doc_attention_mask,
    neg_doc_hidden,
    neg_doc_input_ids,
    neg_doc_attention_mask,
    linear_w,
    linear_b,
):
    fn = _build()
    qh = np.asarray(query_hidden, dtype=np.float32)
    ph = np.asarray(pos_doc_hidden, dtype=np.float32)
    nh = np.asarray(neg_doc_hidden, dtype=np.float32)
    qid = np.asarray(query_input_ids, dtype=np.int32)
    pid = np.asarray(pos_doc_input_ids, dtype=np.int32)
    nid = np.asarray(neg_doc_input_ids, dtype=np.int32)
    qmask = np.asarray(query_attention_mask, dtype=np.int32)
    w = np.asarray(linear_w, dtype=np.float32)
    b = np.asarray(linear_b, dtype=np.float32)

    pos, loss = fn(qh, qid, qmask, ph, pid, nh, nid, w, b)
    pos = np.asarray(pos)          # [B]
    loss = np.asarray(loss)[0]     # loss replicated; all entries identical
    return pos, np.float32(loss)
